# revision 2
# baseline (speedup 1.0000x reference)
"""DenseNGCN layer on 8 Trainium2 NeuronCores — v2.

out = A @ (A @ (X W)) + b, A sparse (1.6M edges, 50k nodes), X [50k,512],
W [512,64].

v2 changes vs baseline:
  - Variable-size slots: a slot is ALL of one dst-row's edges inside a
    128-position gather column (<=32 slots/column), instead of fixed
    groups of 4. Edge positions pack tight: 102400/region vs 122880
    (-16.7% gather descriptors, the bottleneck resource).
  - L1 segment-sum uses per-column [128,32] "value staircase" matrices
    precomputed on host (edge values folded in) and streamed from DRAM
    as matmul stationary operands -> the VectorE edge-value multiply is
    gone.
  - Projected tables stored as bf16 rows padded to 128 ch (256 B, the
    DMA-gather minimum element) -> no f32->bf16 cast of gathered data.
  - Fixed ktile->dst-tile map (position caps 2560/2048 per tile+region)
    keeps the program SPMD-identical across cores; all per-core
    variation is data (idx / svals / rid / permutations).
"""

import dataclasses
import numpy as np

import concourse.bacc as bacc
import concourse.mybir as mybir
import concourse.tile as tile
from concourse.bass_utils import run_bass_kernel_spmd
from concourse.library_config import mlp as mlp_lib

F32 = mybir.dt.float32
BF16 = mybir.dt.bfloat16
I16 = mybir.dt.int16
BF16_NP = mybir.dt.np(BF16)


@dataclasses.dataclass
class Cfg:
    n_nodes: int = 50000
    n_edges: int = 1600000
    in_ch: int = 512
    out_ch: int = 64
    n_cores: int = 8
    n_tiles: int = 49        # dst tiles of 128 rows per core
    n_fat: int = 12          # tiles with 5 ktiles (the rest get 4)
    chunk: int = 4096        # positions per gather chunk (1 psum bank)
    gcall: int = 1024        # positions per dma_gather call
    n_queues: int = 4
    dma_scratch: int = 16384
    iterations: int = 3

    @property
    def r_real(self):
        return self.n_nodes // self.n_cores

    @property
    def r_pad(self):
        return self.n_tiles * 128

    @property
    def ktl_caps(self):      # ktiles (512 positions) per tile per region
        return np.array([5] * self.n_fat + [4] * (self.n_tiles - self.n_fat))

    @property
    def nktl_r(self):
        return int(self.ktl_caps.sum())          # 200

    @property
    def ktl_base(self):
        return np.concatenate([[0], np.cumsum(self.ktl_caps)])

    @property
    def tile_of_ktl(self):
        return np.repeat(np.arange(self.n_tiles), self.ktl_caps)

    @property
    def ep_r(self):          # positions per region
        return self.nktl_r * 512                 # 102400

    @property
    def ep_total(self):
        return 2 * self.ep_r                     # 204800

    @property
    def ncol_r(self):        # gather columns per region
        return self.ep_r // 128                  # 800

    @property
    def n_chunks_r(self):
        return self.ep_r // self.chunk           # 25

    @property
    def region_rows(self):   # table rows per region
        return (self.n_cores // 2) * self.r_pad  # 25088


CFG = Cfg()


# ------------------------------------------------------------------
# host preprocessing
# ------------------------------------------------------------------

def _balance_rows(cnt_a, cnt_b, cfg):
    """Assign local rows to tiles; returns pos[] (row -> tile*128+fill).

    Greedy: rows sorted by total positions desc, placed in the feasible
    tile with most remaining slack. Caps: 128 rows/tile; per-region
    position caps ktl_caps*512.
    """
    nt = cfg.n_tiles
    caps = cfg.ktl_caps * 512
    rows_left = np.full(nt, 128, dtype=np.int64)
    a_left = caps.astype(np.int64).copy()
    b_left = caps.astype(np.int64).copy()
    order = np.argsort(-(cnt_a + cnt_b), kind="stable")
    tile_of = np.full(cfg.r_real, -1, dtype=np.int64)
    for r in order:
        feas = (rows_left > 0) & (a_left >= cnt_a[r]) & (b_left >= cnt_b[r])
        if not feas.any():
            raise RuntimeError("row packing failed; loosen caps")
        slack = np.where(feas,
                         (a_left + b_left) / np.maximum(rows_left, 1), -1.0)
        t = int(np.argmax(slack))
        tile_of[r] = t
        rows_left[t] -= 1
        a_left[t] -= cnt_a[r]
        b_left[t] -= cnt_b[r]
    pos = np.full(cfg.r_real, -1, dtype=np.int64)
    fill = np.zeros(nt, dtype=np.int64)
    for r in range(cfg.r_real):
        t = tile_of[r]
        pos[r] = t * 128 + fill[t]
        fill[t] += 1
    return pos


def preprocess(adj_index, adj_values, cfg=CFG):
    rows = np.asarray(adj_index[0], dtype=np.int64)
    cols = np.asarray(adj_index[1], dtype=np.int64)
    vals = np.asarray(adj_values, dtype=np.float32)
    rr, rp = cfg.r_real, cfg.r_pad
    half = cfg.n_cores // 2
    nkr = cfg.nktl_r

    core_of = rows // rr
    pos_all = []
    edge_data = []
    for c in range(cfg.n_cores):
        m = core_of == c
        rl = rows[m] - c * rr
        cl = cols[m]
        vl = vals[m]
        rg = (cl // rr >= half).astype(np.int64)
        cnt_a = np.bincount(rl[rg == 0], minlength=rr)
        cnt_b = np.bincount(rl[rg == 1], minlength=rr)
        pos = _balance_rows(cnt_a, cnt_b, cfg)
        pos_all.append(pos)
        edge_data.append((rl, cl, vl, rg, cnt_a, cnt_b))

    pos_cat = np.concatenate(pos_all)
    out = []
    for c in range(cfg.n_cores):
        rl, cl, vl, rg, cnt_a, cnt_b = edge_data[c]
        pos = pos_all[c]
        sc = cl // rr
        s_loc = cl % rr
        s_pos = pos_cat[sc * rr + s_loc]
        trow = (sc % half) * rp + s_pos
        assert trow.max() < cfg.region_rows <= 32768

        idx = np.zeros(cfg.ep_total, dtype=np.int16)
        svals = np.zeros((128, 2 * cfg.ncol_r, 32), dtype=np.float32)
        rid = np.full((128, 2 * nkr), -1.0, dtype=np.float32)

        p_of_edge = pos[rl]
        for region in (0, 1):
            cnt = cnt_a if region == 0 else cnt_b
            # position start per packed row position
            cnt_of_pos = np.zeros(rp, dtype=np.int64)
            cnt_of_pos[pos] = cnt
            cpt = cnt_of_pos.reshape(cfg.n_tiles, 128)
            base_in_tile = np.cumsum(cpt, axis=1) - cpt      # [nt,128]
            if ((base_in_tile[:, -1] + cpt[:, -1]) > cfg.ktl_caps * 512).any():
                raise RuntimeError("tile position overflow")
            # region-global start position of each packed row
            row_start = (cfg.ktl_base[:-1, None] * 512
                         + base_in_tile).reshape(-1)

            em = rg == region
            pe = p_of_edge[em]                               # packed dst pos
            # rank of edge within its dst row (stable)
            o = np.argsort(pe, kind="stable")
            pe_s = pe[o]
            first = np.searchsorted(pe_s, pe_s)
            rank = np.empty(pe.size, dtype=np.int64)
            rank[o] = np.arange(pe_s.size) - first

            e = row_start[pe] + rank                         # region position
            assert e.max() < cfg.ep_r
            col = e // 128
            p = e % 128
            # slot of (row, col): rank of the segment within the column.
            # Segments in a column are ordered by start position == by row
            # start; rows are disjoint ranges, so segment rank = number of
            # distinct rows in this column before this row.
            seg_key = col * rp + pe                          # unique per (col,row)
            uniq, inv = np.unique(seg_key, return_inverse=True)
            ucol = uniq // rp
            # rank within column (uniq sorted by (col, row-pos))
            ufirst = np.searchsorted(ucol, ucol)
            useg = np.arange(uniq.size) - ufirst
            assert useg.max() < 32, f"segment overflow {useg.max()}"
            s = useg[inv]

            idx[cfg.ep_r * region + e] = trow[em].astype(np.int16)
            svals[p, cfg.ncol_r * region + col, s] = vl[em]

            # rid: (col, slot) -> owner row (pos within tile)
            upart = 32 * (ucol % 4) + useg
            uktl = ucol // 4
            uowner = (uniq % rp) % 128                       # pe % 128
            rid[upart, nkr * region + uktl] = uowner.astype(np.float32)

        out.append(dict(
            idx=np.tile(idx.reshape(-1, 16).T, (8, 1)).copy(),
            svals=np.ascontiguousarray(
                svals.astype(BF16_NP)),
            rid=rid))
    return out, pos_all


# ------------------------------------------------------------------
# device program
# ------------------------------------------------------------------

def _bc_last(ap, n):
    return dataclasses.replace(ap, ap=list(ap.ap) + [[0, n]])


def build_program(cfg=CFG):
    nc = bacc.Bacc(None, target_bir_lowering=False, debug=False,
                   num_swdge_queues=cfg.n_queues,
                   dynamic_dma_scratch_size=cfg.dma_scratch)
    rp, nt = cfg.r_pad, cfg.n_tiles
    nkr, ep_r, ch = cfg.nktl_r, cfg.ep_r, cfg.chunk
    ncol_r = cfg.ncol_r
    kc = cfg.in_ch // 128
    cpc = ch // 128                    # columns per chunk (32)
    TC = 128                           # padded channel count (bf16, 256B)

    featT_d = nc.declare_dram_parameter("featT", [cfg.in_ch, rp], BF16, isOutput=False)
    w_d = nc.declare_dram_parameter("w", [cfg.in_ch, cfg.out_ch], BF16, isOutput=False)
    idx_d = nc.declare_dram_parameter("idx", [128, cfg.ep_total // 16], I16, isOutput=False)
    svals_d = nc.declare_dram_parameter("svals", [128, 2 * ncol_r, 32], BF16, isOutput=False)
    rid_d = nc.declare_dram_parameter("rid", [128, 2 * nkr], F32, isOutput=False)
    iota_d = nc.declare_dram_parameter("iota", [128, 128], F32, isOutput=False)
    bias_d = nc.declare_dram_parameter("biasr", [128, cfg.out_ch], F32, isOutput=False)
    out_d = nc.declare_dram_parameter("out", [rp, cfg.out_ch], F32, isOutput=True)

    shard = [nc.dram_tensor(f"shard{i}", [rp, TC], BF16) for i in range(2)]
    table = [nc.dram_tensor(f"table{i}", [cfg.region_rows * 2, TC], BF16,
                            addr_space="Shared") for i in range(2)]
    groups = [list(range(cfg.n_cores))]

    tile_of_ktl = cfg.tile_of_ktl
    ktl_base = cfg.ktl_base

    with tile.TileContext(nc) as tc:
        with tc.tile_pool(name="const", bufs=1) as constp:
            nc.gpsimd.load_library(mlp_lib)
            iota = constp.tile([128, 128], F32)
            nc.sync.dma_start(iota[:], iota_d[:])
            rid = constp.tile([128, 2 * nkr], F32)
            nc.sync.dma_start(rid[:], rid_d[:])
            idx = constp.tile([128, cfg.ep_total // 16], I16)
            nc.sync.dma_start(idx[:], idx_d[:])
            bias = constp.tile([128, cfg.out_ch], F32)
            nc.sync.dma_start(bias[:], bias_d[:])

            # ---------------- XW ----------------
            with (
                tc.tile_pool(name="feat", bufs=1) as featp,
                tc.tile_pool(name="xwps", bufs=2, space="PSUM") as xwps,
                tc.tile_pool(name="stg", bufs=1) as stgp,
            ):
                feat = featp.tile([128, kc, rp], BF16)
                nc.sync.dma_start(
                    feat[:], featT_d[:].rearrange("(a p) n -> p a n", p=128))
                wsb = featp.tile([128, kc, cfg.out_ch], BF16)
                nc.sync.dma_start(
                    wsb[:], w_d[:].rearrange("(a p) f -> p a f", p=128))
                stg1 = stgp.tile([128, nt, TC], BF16)
                nc.vector.memset(stg1[:], 0.0)
                for t in range(nt):
                    ps = xwps.tile([128, cfg.out_ch], F32, tag="xw", name=f"xw{t}")
                    for a in range(kc):
                        nc.tensor.matmul(
                            ps[:], feat[:, a, t * 128:(t + 1) * 128],
                            wsb[:, a, :], start=(a == 0), stop=(a == kc - 1))
                    nc.scalar.copy(stg1[:, t, 0:cfg.out_ch], ps[:])
                nc.sync.dma_start(
                    shard[0][:].rearrange("(t p) f -> p t f", p=128), stg1[:])
            nc.gpsimd.collective_compute(
                "AllGather", mybir.AluOpType.bypass,
                ins=[shard[0][:]], outs=[table[0][:]], replica_groups=groups)

            # ---------------- two SPMM iterations ----------------
            for it in range(cfg.iterations - 1):
                last = it == cfg.iterations - 2
                with (
                    tc.tile_pool(name=f"g{it}", bufs=4) as gpool,
                    tc.tile_pool(name=f"sv{it}", bufs=3) as svpool,
                    tc.tile_pool(name=f"srs{it}", bufs=4) as srspool,
                    tc.tile_pool(name=f"oh{it}", bufs=2) as ohpool,
                    tc.tile_pool(name=f"stg{it}", bufs=1) as stgp,
                    tc.tile_pool(name=f"l1ps{it}", bufs=3, space="PSUM") as l1ps,
                    tc.tile_pool(name=f"l2ps{it}", bufs=4, space="PSUM") as l2ps,
                ):
                    stg = stgp.tile([128, nt, cfg.out_ch], F32,
                                    name=f"stg_{it}")
                    if not last:
                        stgbf = stgp.tile([128, nt, TC], BF16,
                                          name=f"stgbf_{it}")
                        nc.vector.memset(stgbf[:], 0.0)
                    tbl = table[it]
                    qn = [0]

                    for region in range(2):
                        tbl_ap = (tbl[0:cfg.region_rows, :] if region == 0
                                  else tbl[cfg.region_rows:2 * cfg.region_rows, :])
                        l2acc = {}
                        mm_done = [0] * nt
                        mm_total = list(cfg.ktl_caps)
                        for chk in range(cfg.n_chunks_r):
                            g = gpool.tile([128, cpc, TC], BF16, tag="g",
                                           name=f"g_{it}_{region}_{chk}")
                            ncall = ch // cfg.gcall
                            gct = cfg.gcall // 128
                            for ci in range(ncall):
                                i0 = (region * ep_r + chk * ch
                                      + ci * cfg.gcall) // 16
                                nc.gpsimd.dma_gather(
                                    g[:, ci * gct:(ci + 1) * gct, :], tbl_ap,
                                    idx[:, i0:i0 + cfg.gcall // 16],
                                    cfg.gcall, cfg.gcall, TC,
                                    queue_num=qn[0] % cfg.n_queues)
                                qn[0] += 1
                            sv = svpool.tile([128, cpc, 32], BF16, tag="sv",
                                             name=f"sv_{it}_{region}_{chk}")
                            c0 = region * ncol_r + chk * cpc
                            nc.sync.dma_start(sv[:], svals_d[:, c0:c0 + cpc, :])
                            # L1: one matmul per column
                            ps = l1ps.tile([128, 512], F32, tag="l1",
                                           name=f"l1_{it}_{region}_{chk}")
                            for c in range(cpc):
                                nc.tensor.matmul(
                                    ps[32 * (c % 4):32 * (c % 4) + 32,
                                       64 * (c // 4):64 * (c // 4) + 64],
                                    sv[:, c, :], g[:, c, 0:cfg.out_ch],
                                    start=True, stop=True,
                                    tile_position=(0, 32 * (c % 4)))
                            srs = srspool.tile([128, 512], BF16, tag="srs",
                                               name=f"srs_{it}_{region}_{chk}")
                            nc.scalar.copy(srs[:], ps[:])
                            # L2: 8 ktiles per chunk
                            oh = ohpool.tile([128, 8, 128], BF16, tag="oh",
                                             name=f"oh_{it}_{region}_{chk}")
                            kg0 = region * nkr + chk * 8
                            nc.vector.tensor_tensor(
                                oh[:], _bc_last(rid[:, kg0:kg0 + 8], 128),
                                dataclasses.replace(
                                    iota[:],
                                    ap=[iota[:].ap[0], [0, 8], iota[:].ap[1]]),
                                mybir.AluOpType.is_equal)
                            for cc in range(8):
                                a = chk * 8 + cc
                                t = int(tile_of_ktl[a])
                                if t not in l2acc:
                                    l2acc[t] = l2ps.tile(
                                        [128, cfg.out_ch], F32, tag="l2acc",
                                        name=f"l2acc_{it}_{region}_{t}")
                                nc.tensor.matmul(
                                    l2acc[t][:], oh[:, cc, :],
                                    srs[:, 64 * cc:64 * cc + 64],
                                    start=(mm_done[t] == 0),
                                    stop=(mm_done[t] == mm_total[t] - 1))
                                mm_done[t] += 1
                                if mm_done[t] == mm_total[t]:
                                    if region == 0:
                                        nc.scalar.copy(
                                            stg[:, t, :], l2acc[t][:])
                                    else:
                                        nc.vector.tensor_add(
                                            stg[:, t, :], stg[:, t, :],
                                            l2acc[t][:])
                                        if last:
                                            nc.vector.tensor_add(
                                                stg[:, t, :], stg[:, t, :],
                                                bias[:])
                                        else:
                                            nc.scalar.copy(
                                                stgbf[:, t, 0:cfg.out_ch],
                                                stg[:, t, :])
                                    del l2acc[t]
                        assert not l2acc

                    if last:
                        nc.sync.dma_start(
                            out_d[:].rearrange("(t p) f -> p t f", p=128),
                            stg[:])
                    else:
                        nc.sync.dma_start(
                            shard[1][:].rearrange("(t p) f -> p t f", p=128),
                            stgbf[:])
                if not last:
                    nc.gpsimd.collective_compute(
                        "AllGather", mybir.AluOpType.bypass,
                        ins=[shard[1][:]], outs=[table[1][:]],
                        replica_groups=groups)

    nc.compile()
    return nc


# ------------------------------------------------------------------
# host-side input/output marshalling
# ------------------------------------------------------------------

def make_in_maps(inputs, pre, pos_all, cfg=CFG):
    feats = np.asarray(inputs["features"], dtype=np.float32)
    wm = np.asarray(inputs["weight_matrix"], dtype=np.float32)
    bias = np.asarray(inputs["bias"], dtype=np.float32)
    iota = np.tile(np.arange(128, dtype=np.float32), (128, 1))
    bias_rep = np.tile(bias.reshape(1, cfg.out_ch), (128, 1)).astype(np.float32)
    w_bf = wm.astype(BF16_NP)
    in_maps = []
    for c in range(cfg.n_cores):
        fc = feats[c * cfg.r_real:(c + 1) * cfg.r_real]
        fp = np.zeros((cfg.r_pad, cfg.in_ch), dtype=np.float32)
        fp[pos_all[c]] = fc
        in_maps.append(dict(
            featT=np.ascontiguousarray(fp.T).astype(BF16_NP),
            w=w_bf, idx=pre[c]["idx"], svals=pre[c]["svals"],
            rid=pre[c]["rid"], iota=iota, biasr=bias_rep))
    return in_maps


_CACHE = {}


def kernel(adj_index, adj_values, features, weight_matrix, bias):
    cfg = CFG
    key = "prog"
    if key not in _CACHE:
        _CACHE[key] = build_program(cfg)
    nc = _CACHE[key]
    pre, pos_all = preprocess(adj_index, adj_values, cfg)
    in_maps = make_in_maps(
        dict(features=features, weight_matrix=weight_matrix, bias=bias),
        pre, pos_all, cfg)
    res = run_bass_kernel_spmd(nc, in_maps, core_ids=list(range(cfg.n_cores)))
    out = np.zeros((cfg.n_nodes, weight_matrix.shape[1]), dtype=np.float32)
    for c in range(cfg.n_cores):
        shard = res.results[c]["out"]
        out[c * cfg.r_real:(c + 1) * cfg.r_real] = shard[pos_all[c]]
    return out
